# revision 9
# baseline (speedup 1.0000x reference)
"""Trainium2 Bass kernel for a species-routed MoE readout layer.

Math (see problem reference): per atom x [512]:
  u = silu(emb[species]); scores = softmax(u @ Wr.T)  -> top-2 sparse gates
  out = sum_e gate_e * (W2_e @ silu(W1_e @ x + b1_e) + b2_e)
      + sum_s (W2_s @ silu(W1_s @ x + b1_s) + b2_s)          # 2 shared experts

The router depends only on species_idx (64 species), so the per-atom top-2
gates collapse to a host-computed 64x6 lookup table. Atoms are grouped by
their top-2 expert pair and each group is split evenly across the 8 cores so
the single SPMD program sees the same tile->active-expert pattern on every
core; interior tiles then only compute 2 routed + 2 shared expert MLPs
instead of all 8.

Precision split (the router logits are tiny, so every softmax gate is
~1/6: routed-expert outputs are damped 6x while the 2 shared experts pass
at full weight): the 2 routed experts run in fp8-e4m3 with DoubleRow
double-pumped matmuls (2 contraction planes per PE pass, ~2x bf16), the
2 shared experts stay bf16. Measured end-to-end max-rel error ~9e-3 vs
the 2e-2 budget (all-bf16 is 3.4e-3; all-fp8 would be 5e-2).

Scale plumbing (e4m3 normals bottom out at 2^-6, weights are ~0.02):
  W1q = e4m3(32*W1)  -> mm1 psum = 32*pre; silu applied as
        ScalarE activation(scale=1/32, bias=b1)
  hpm = e4m3((h + alpha_e) * 8w)  on DVE STT (w6 carries 8*gate;
        alpha_e = lstsq(W2_e, b2_e) folds the gated b2 into the gate mult)
  W2q = e4m3(32*W2)  -> routed psum2 terms = 256 * w*(W2_e(h+alpha))
  shared W2 (bf16) pre-scaled x256 to match; final PSUM->SBUF copy is
  (psum * 1/256) + sum_s b2_s on the DVE tensor_scalar.

Device side (per core, per <=512-atom variable-size tile aligned to
expert-pair segment boundaries), f32 PSUM accumulation throughout.
ScalarE's ACTIVATE carries a 352-cycle fixed cost, so each expert's
hidden units are host-permuted to pair near-equal b1 values into partner
chunks (2P, 2P+1): one silu activation then covers a 2-bank PSUM pair
with the shared pair-mean bias (approximation error ~3e-4 max-rel),
halving ScalarE's instruction count. mm2 blocks are software-pipelined
three expert-blocks behind mm1 so the in-order PE queue never waits on
the silu->gate-mult chain. A short PE spin + dummy activation at kernel
start warms the HAM clock gate (cold PE runs at 1.2 instead of 2.4 GHz)
and the ACT table while the first DMAs are in flight; per-expert weight
tiles stream on the sync queue in first-use order behind tile 0's
inputs, gate rows/consts/broadcasts ride the GPSIMD queue.
"""

import numpy as np
import ml_dtypes

import concourse.bass as bass
import concourse.mybir as mybir
from concourse import bacc, tile
from concourse.bass_utils import run_bass_kernel_spmd

BF16 = mybir.dt.bfloat16
FP8 = mybir.dt.float8e4
F32 = mybir.dt.float32
BF16_NP = ml_dtypes.bfloat16
FP8_NP = ml_dtypes.float8_e4m3  # IEEE e4m3 (max 240) == TRN FP8_EXP4

N_CORES = 8
N_ATOMS = 100000
IN_F = 512
HID = 512
OUT_F = 256
N_ROUTED = 6
N_SHARED = 2
N_EXP = N_ROUTED + N_SHARED
TOPK = 2
TILE_N = 512  # atoms per tile = one PSUM bank = max matmul moving dim
KC = IN_F // 128   # 4 contraction chunks for mm1
MC = HID // 128    # 4 hid chunks
OC = OUT_F // 128  # 2 out chunks

SCALE_W = 32.0           # fp8 weight scale (W1q and W2q)
SCALE_G = 8.0            # gate scale folded into the hpm quantization
SCALE_OUT = SCALE_W * SCALE_G  # routed+shared psum arrives x256

SPARSE = True  # compute only active routed experts per tile
ROUTED_FP8 = True  # routed experts in fp8 DoubleRow; False = all-bf16
# CoreSim has no Silu: decompose as x*sigmoid(x) (bias folded into PSUM via a
# rank-1 matmul). HW path uses native ScalarE Silu with the f32 bias operand.
SILU_DECOMP = False
# Timing-only CoreSim mode: emit the HW-path instruction stream but with
# Sigmoid in place of Silu (identical cost shape, wrong values).
SILU_AS_SIGMOID = False

DR = mybir.MatmulPerfMode.DoubleRow


def _silu(x):
    return x / (1.0 + np.exp(-x))


def _router_table(emb, W_router):
    """[64, 6] sparse top-2 gate table + per-species expert pair."""
    u = _silu(emb.astype(np.float32))
    logits = u @ W_router.astype(np.float32).T
    m = logits.max(axis=-1, keepdims=True)
    e = np.exp(logits - m)
    scores = e / e.sum(axis=-1, keepdims=True)
    order = np.argsort(-scores, axis=-1, kind="stable")
    top2 = order[:, :TOPK]
    wt = np.zeros_like(scores)
    rows = np.arange(scores.shape[0])[:, None]
    wt[rows, top2] = scores[rows, top2]
    return wt, top2


def _plan_sharding(species_idx, top2):
    """Group atoms by top-2 expert pair, split each group evenly over cores.

    Returns (idx_cores [N_CORES, NL] int64 with -1 padding, tiles) where
    tiles is a list of (n_atoms, active_routed_experts) per tile,
    identical for every core by construction.
    """
    n = species_idx.shape[0]
    if not SPARSE:
        assert n % N_CORES == 0
        nl = n // N_CORES
        idx_cores = np.arange(n, dtype=np.int64).reshape(N_CORES, nl)
        tiles = []
        for t0 in range(0, nl, TILE_N):
            tiles.append((min(TILE_N, nl - t0), tuple(range(N_ROUTED))))
        return idx_cores, tiles

    MIN_TILE = 64  # merge segments smaller than this into their neighbor

    pair_of_species = [tuple(sorted(top2[s])) for s in range(top2.shape[0])]
    pairs = sorted(set(pair_of_species))
    pair_id_of_species = np.array(
        [pairs.index(p) for p in pair_of_species], dtype=np.int64
    )
    atom_pair = pair_id_of_species[species_idx]

    seg_lens = []       # per-group per-core segment length
    seg_experts = []
    group_idx = []      # per-group atom index arrays
    for g, p in enumerate(pairs):
        idx_g = np.nonzero(atom_pair == g)[0]
        if idx_g.size == 0:
            continue
        L = -(-idx_g.size // N_CORES)  # ceil
        seg_lens.append(L)
        seg_experts.append(tuple(int(x) for x in p))
        group_idx.append(idx_g)

    # largest group first: deep pipeline while the clock warms, short tail
    order = np.argsort([-L for L in seg_lens], kind="stable")
    seg_lens = [seg_lens[i] for i in order]
    seg_experts = [seg_experts[i] for i in order]
    group_idx = [group_idx[i] for i in order]

    nl = sum(seg_lens)
    idx_cores = np.full((N_CORES, nl), -1, dtype=np.int64)
    off = 0
    for L, idx_g in zip(seg_lens, group_idx):
        for c in range(N_CORES):
            part = idx_g[c * L : (c + 1) * L]
            idx_cores[c, off : off + part.size] = part
        off += L

    # Variable-size tiles aligned to segment boundaries: each tile covers a
    # single expert pair (tiny segments merge into their neighbor).
    tiles = []
    pend_n, pend_e = 0, set()
    for L, p in zip(seg_lens, seg_experts):
        pend_n += L
        pend_e.update(p)
        if pend_n < MIN_TILE:
            continue
        k = -(-pend_n // TILE_N)
        q, r = divmod(pend_n, k)
        for i in range(k):
            tiles.append((q + (1 if i < r else 0), tuple(sorted(pend_e))))
        pend_n, pend_e = 0, set()
    if pend_n:
        if tiles:
            n0, e0 = tiles.pop()
            pend_n += n0
            pend_e.update(e0)
        k = -(-pend_n // TILE_N)
        q, r = divmod(pend_n, k)
        ee = tuple(sorted(pend_e))
        for i in range(k):
            tiles.append((q + (1 if i < r else 0), ee))
    assert sum(t[0] for t in tiles) == nl
    return idx_cores, tiles


def _build_program(nl, tiles):
    nc = bacc.Bacc("TRN2", target_bir_lowering=False, debug=False)

    # x/xq/w6/out are TILE-PACKED on the host: per tile a contiguous
    # [128, KC*n] (resp. [1, r*n] / [128, OC*n]) block, so every stream
    # DMA is one big contiguous transfer instead of 512 sub-KB rows.
    tot_x = sum(KC * n for n, _ in tiles)
    tot_w6 = sum(len(r) * n for n, r in tiles)
    tot_out = sum(OC * n for n, _ in tiles)
    xT_d = nc.declare_dram_parameter("xT", [128, tot_x], BF16, isOutput=False)
    if ROUTED_FP8:
        xqT_d = nc.declare_dram_parameter("xqT", [128, tot_x], FP8, isOutput=False)
    w6_d = nc.declare_dram_parameter("w6", [1, tot_w6], BF16, isOutput=False)
    rw_dt = FP8 if ROUTED_FP8 else BF16
    # weights contiguous per expert ([e][p][k][cols]) so the first
    # critical-path loads run at full DMA bandwidth
    w1q_d = nc.declare_dram_parameter("w1q", [N_ROUTED, 128, KC, HID], rw_dt, isOutput=False)
    w2q_d = nc.declare_dram_parameter("w2q", [N_ROUTED, 128, MC, OUT_F], rw_dt, isOutput=False)
    w1s_d = nc.declare_dram_parameter("w1s", [N_SHARED, 128, KC, HID], BF16, isOutput=False)
    w2s_d = nc.declare_dram_parameter("w2s", [N_SHARED, 128, MC, OUT_F], BF16, isOutput=False)
    b1_d = nc.declare_dram_parameter("b1", [128, N_EXP * (MC // 2)], F32, isOutput=False)
    b1r_d = nc.declare_dram_parameter("b1r", [1, N_EXP * HID], BF16, isOutput=False)
    # alpha[e] solves W2_e @ alpha_e = b2_e (host lstsq), so the gated b2
    # rides the gate multiply: W2_e @ (w*(h+alpha)) = w*(W2_e h) + w*b2_e
    alpha_d = nc.declare_dram_parameter(
        "alpha", [128, N_ROUTED * MC], F32, isOutput=False
    )
    b2s_d = nc.declare_dram_parameter("b2s", [128, OC], F32, isOutput=False)
    outT_d = nc.declare_dram_parameter("outT", [128, tot_out], BF16, isOutput=True)

    mm1_scale = 1.0 / SCALE_W if ROUTED_FP8 else 1.0

    with tile.TileContext(nc) as tc:
        with (
            tc.tile_pool(name="consts", bufs=1) as consts,
            tc.tile_pool(name="xp", bufs=6) as xp,
            tc.tile_pool(name="xqp", bufs=6) as xqp,
            tc.tile_pool(name="w6p", bufs=3) as w6p,
            tc.tile_pool(name="wbcp", bufs=6) as wbcp,
            tc.tile_pool(name="hps", bufs=2, space="PSUM") as hpsp,
            tc.tile_pool(name="hp", bufs=10) as hp_pool,
            tc.tile_pool(name="hpp", bufs=6) as hpp_pool,
            tc.tile_pool(name="ops", bufs=4, space="PSUM") as outps_pool,
            tc.tile_pool(name="osb", bufs=5) as osb_pool,
        ):
            # ---- constants / weights preload ----
            # Queue split: bulk streams (x/xq + expert weights) on the sync
            # queue; small/slow-lane transfers (per-partition consts, the
            # single-partition w6 gate rows) plus gate broadcasts and output
            # writes on the GPSIMD queue. Keeps tile0's first mm1 deps (w1[e0]
            # + xq) arriving ~1.7us in, right as the HAM warm-up spin ends.
            b1_sb = consts.tile([128, N_EXP * (MC // 2)], F32, name="b1_sb")
            alpha_sb = consts.tile([128, N_ROUTED * MC], F32, name="alpha_sb")
            b2s_sb = consts.tile([128, OC], F32, name="b2s_sb")
            ones_sb = consts.tile([1, 128], BF16, name="ones_sb")
            ones_row = consts.tile([1, TILE_N], BF16, name="ones_row")

            nc.gpsimd.dma_start(b1_sb[:], b1_d[:])
            nc.gpsimd.dma_start(alpha_sb[:], alpha_d[:])
            nc.gpsimd.dma_start(b2s_sb[:], b2s_d[:])
            if SILU_DECOMP:
                # dead in the HW path; its single-partition row is a ~3us
                # slow-lane DMA, so only load it for the CoreSim decomp
                b1row_sb = consts.tile([1, N_EXP * HID], BF16, name="b1row_sb")
                nc.gpsimd.dma_start(b1row_sb[:], b1r_d[:])
            nc.vector.memset(ones_sb[:], 1.0)
            nc.vector.memset(ones_row[:], 1.0)

            # Per-expert weight tiles, loaded in first-use order so tile 0's
            # matmuls start after the first expert's ~0.4MB instead of the
            # full weight set.
            eorder = []
            for _, routed in tiles:
                for e in list(routed) + [N_ROUTED + s for s in range(N_SHARED)]:
                    if e not in eorder:
                        eorder.append(e)
                if len(eorder) == N_EXP:
                    break
            for e in range(N_EXP):
                if e not in eorder:
                    eorder.append(e)

            # one contiguous DMA per expert per matrix (queue-issue cost is
            # per-op, transfers run on the parallel DMA engines)
            w1_sb = {}
            w2_sb = {}

            def load_w1(e):
                dt = rw_dt if e < N_ROUTED else BF16
                w1_sb[e] = consts.tile([128, KC, HID], dt, name=f"w1e{e}")
                src = w1q_d[e] if e < N_ROUTED else w1s_d[e - N_ROUTED]
                nc.sync.dma_start(w1_sb[e][:], src)

            def load_w2(e):
                dt = rw_dt if e < N_ROUTED else BF16
                w2_sb[e] = consts.tile([128, MC, OUT_F], dt, name=f"w2e{e}")
                src = w2q_d[e] if e < N_ROUTED else w2s_d[e - N_ROUTED]
                nc.sync.dma_start(w2_sb[e][:], src)

            # only w1[e0] ahead of tile0's xq: first mm1 needs exactly these
            load_w1(eorder[0])

            # Warm the PE HAM clock gate (cold = 1.2 GHz until ~3.4us of
            # sustained activity) and the ScalarE activation table while the
            # first input DMAs are in flight.
            warm_sb = consts.tile([128, 1], F32, name="warm_sb")
            for _ in range(8):
                warm_ps = hpsp.tile(
                    [128, 2, TILE_N], F32, name="warm_ps", tag="hps"
                )
                nc.tensor.matmul(
                    warm_ps[:, 0, :], ones_sb[:, :], ones_row[0:1, :],
                    start=True, stop=True,
                )
            nc.scalar.activation(
                warm_sb[:, :], b1_sb[:, 0:1],
                mybir.ActivationFunctionType.Sigmoid
                if (SILU_DECOMP or SILU_AS_SIGMOID)
                else mybir.ActivationFunctionType.Silu,
            )

            # ---- main loop over atom tiles (tile-packed dram offsets) ----
            xoff = 0
            woff = 0
            ooff = 0

            # Software pipelining: each expert's mm2 (and, for a tile's last
            # expert, the psum->sbuf tail) is emitted three expert-blocks
            # late (flush keeps 2 pending), so the in-order PE queue never
            # stalls on the ScalarE silu -> DVE gate-mult chain of the last
            # m-chunk (one block of fp8 mm1 is shorter than that latency).
            pending = []

            def flush_pending(keep=0):
                while len(pending) > keep:
                    pending.pop(0)()

            a0 = 0
            spare_experts = []
            for t, (n, routed) in enumerate(tiles):
                # alternate routed (fp8, short mm1 bursts) with shared (bf16,
                # long mm1) so ScalarE's silu drain keeps up with PSUM-bank
                # production; tile0 runs routed first so its startup deps are
                # the cheap fp8 streams (xq + fp8 w1) only
                rr = list(routed)
                ss = [N_ROUTED + s for s in range(N_SHARED)]
                if t == 0:
                    experts = rr + ss
                else:
                    experts = []
                    while rr or ss:
                        if rr:
                            experts.append(rr.pop(0))
                        if ss:
                            experts.append(ss.pop(0))
                # safety net: a tile must never route to an unloaded expert
                for e in experts:
                    if t > 0 and e not in w1_sb:
                        if e in spare_experts:
                            spare_experts.remove(e)
                        load_w1(e)
                        load_w2(e)

                # xq first: the routed experts run first and only need xq+gates
                nr = len(routed)
                if ROUTED_FP8:
                    xq_sb = xqp.tile([128, KC, n], FP8, name="xq_sb", tag="xq")
                    nc.sync.dma_start(
                        xq_sb[:], xqT_d[:, xoff : xoff + KC * n]
                    )
                if t == 0:
                    # tile0's first two experts' mm2 weights + second expert's
                    # w1 right behind xq, ahead of the bf16 x stream, so the
                    # software-pipelined first mm2 never stalls on its DMA
                    load_w2(experts[0])
                    load_w1(experts[1])
                    load_w2(experts[1])
                # gate rows packed onto partition 0 (single-partition DMAs are
                # the slow per-partition lane -> GPSIMD queue; one contiguous
                # block with this tile's routed rows only)
                w6row = w6p.tile([1, nr, n], BF16, name="w6row", tag="w6r")
                nc.gpsimd.dma_start(w6row[:], w6_d[0:1, woff : woff + nr * n])
                x_sb = xp.tile([128, KC, n], BF16, name="x_sb", tag="x")
                nc.sync.dma_start(x_sb[:], xT_d[:, xoff : xoff + KC * n])

                if t == 0:
                    # tile0's own remaining weights right behind its inputs;
                    # spare experts trickle one per later tile so they never
                    # sit ahead of the next tiles' x/xq streams
                    for e in experts[2:]:
                        if e not in w1_sb:
                            load_w1(e)
                        if e not in w2_sb:
                            load_w2(e)
                    spare_experts.extend(
                        e for e in eorder if e not in w1_sb
                    )
                elif spare_experts:
                    e_sp = spare_experts.pop(0)
                    if e_sp not in w1_sb:
                        load_w1(e_sp)
                        load_w2(e_sp)

                # per-atom gates broadcast across 128 partitions (GPSIMD,
                # keeps PE free)
                wsb = {}
                for si, e in enumerate(routed):
                    wsb_e = wbcp.tile([128, TILE_N], BF16, name="wsb", tag="wbc")
                    nc.gpsimd.partition_broadcast(
                        wsb_e[:, :n], w6row[0:1, si, :n]
                    )
                    wsb[e] = wsb_e

                # output accumulators
                outps = [
                    outps_pool.tile([128, TILE_N], F32, name="ops", tag="ops")
                    for _ in range(OC)
                ]

                for ei, e in enumerate(experts):
                    is_routed = e < N_ROUTED
                    use_fp8 = ROUTED_FP8 and is_routed
                    # DoubleRow only beats plain matmul when the stream
                    # (1.13*n/2 cyc/chunk) outruns its 256-col LDW; below
                    # n~256 use mixed fp8-weight x bf16-x matmuls instead
                    # (same fp8 weights and scales, bf16 gate path)
                    use_dr = use_fp8 and n >= 256
                    last_e = ei == len(experts) - 1
                    # flush the 3-blocks-ago mm2 BEFORE this expert's mm1:
                    # under the routed/shared interleave the flushed block is
                    # the opposite precision, so DoubleRow bursts (whose
                    # LDWEIGHTS chain outpaces their streams) are separated
                    # by bf16 work that lets the weight port drain
                    flush_pending(keep=2)
                    hpm8 = None
                    if use_dr:
                        hpm8 = hpp_pool.tile(
                            [128, MC, TILE_N], FP8, name="hpm8", tag="hpm8"
                        )
                    hpm_list = []  # (pair_tile, r) mm2 rhs per m-chunk (bf16)
                    for P in range(MC // 2):
                        # 2-chunk PSUM pair: partner chunks 2P/2P+1 share a
                        # per-partition bias (host pairs near-equal b1), so
                        # ONE ScalarE activation covers both banks.
                        hps = hpsp.tile(
                            [128, 2, TILE_N], F32, name="hps", tag="hps"
                        )
                        for r in range(2):
                            m = 2 * P + r
                            if use_dr:
                                for kk in range(0, KC, 2):
                                    nc.tensor.matmul(
                                        hps[:, r, :n],
                                        w1_sb[e][:, kk : kk + 2, m * 128 : (m + 1) * 128],
                                        xq_sb[:, kk : kk + 2, :n],
                                        start=(kk == 0),
                                        stop=(kk == KC - 2 and not SILU_DECOMP),
                                        perf_mode=DR,
                                    )
                            else:
                                for k in range(KC):
                                    nc.tensor.matmul(
                                        hps[:, r, :n],
                                        w1_sb[e][:, k, m * 128 : (m + 1) * 128],
                                        x_sb[:, k, :n],
                                        start=(k == 0),
                                        stop=(k == KC - 1 and not SILU_DECOMP),
                                    )
                            if SILU_DECOMP:
                                boff = e * HID + m * 128
                                nc.tensor.matmul(
                                    hps[:, r, :n],
                                    b1row_sb[0:1, boff : boff + 128],
                                    ones_row[0:1, :n],
                                    start=False, stop=True,
                                )
                        h_sb = hp_pool.tile(
                            [128, 2, TILE_N], BF16, name="h_sb", tag="h"
                        )
                        sc = mm1_scale if use_fp8 else 1.0
                        if SILU_DECOMP:
                            # bias is already in psum (rank-1, pair-mean b1);
                            # h_sb ends up as sc^-1 * h, absorbed by STT/w2
                            s_sb = hp_pool.tile(
                                [128, 2, TILE_N], BF16, name="s_sb", tag="s"
                            )
                            nc.scalar.activation(
                                s_sb[:, :, :n], hps[:, :, :n],
                                mybir.ActivationFunctionType.Sigmoid,
                                scale=sc,
                            )
                            nc.vector.tensor_mul(
                                h_sb[:, :, :n], hps[:, :, :n], s_sb[:, :, :n]
                            )
                        else:
                            nc.scalar.activation(
                                h_sb[:, :, :n], hps[:, :, :n],
                                mybir.ActivationFunctionType.Sigmoid
                                if SILU_AS_SIGMOID
                                else mybir.ActivationFunctionType.Silu,
                                bias=b1_sb[:, e * (MC // 2) + P : e * (MC // 2) + P + 1],
                                scale=sc,
                            )
                        for r in range(2):
                            m = 2 * P + r
                            if is_routed:
                                ac = e * MC + m
                                if use_dr:
                                    nc.vector.scalar_tensor_tensor(
                                        hpm8[:, m, :n],
                                        h_sb[:, r, :n],
                                        alpha_sb[:, ac : ac + 1],
                                        wsb[e][:, :n],
                                        mybir.AluOpType.add,
                                        mybir.AluOpType.mult,
                                    )
                                else:
                                    hpm = hpp_pool.tile(
                                        [128, TILE_N], BF16, name="hpm", tag="hpm"
                                    )
                                    nc.vector.scalar_tensor_tensor(
                                        hpm[:, :n],
                                        h_sb[:, r, :n],
                                        alpha_sb[:, ac : ac + 1],
                                        wsb[e][:, :n],
                                        mybir.AluOpType.add,
                                        mybir.AluOpType.mult,
                                    )
                                    hpm_list.append((hpm, None))
                            else:
                                hpm_list.append((h_sb, r))

                    def emit_mm2(
                        e=e, ei=ei, n=n, last_e=last_e, use_dr=use_dr,
                        hpm8=hpm8, hpm_list=tuple(hpm_list), outps=tuple(outps),
                    ):
                        if use_dr:
                            for c in range(OC):
                                for mm in range(0, MC, 2):
                                    nc.tensor.matmul(
                                        outps[c][:, :n],
                                        w2_sb[e][:, mm : mm + 2, c * 128 : (c + 1) * 128],
                                        hpm8[:, mm : mm + 2, :n],
                                        start=(ei == 0 and mm == 0),
                                        stop=(last_e and mm == MC - 2),
                                        perf_mode=DR,
                                    )
                        else:
                            for c in range(OC):
                                for m in range(MC):
                                    ht, hr = hpm_list[m]
                                    rhs = ht[:, :n] if hr is None else ht[:, hr, :n]
                                    nc.tensor.matmul(
                                        outps[c][:, :n],
                                        w2_sb[e][:, m, c * 128 : (c + 1) * 128],
                                        rhs,
                                        start=(ei == 0 and m == 0),
                                        stop=(last_e and m == MC - 1),
                                    )

                    # psum -> sbuf on DVE: (psum * 1/256) + sum_s b2_s, cast
                    # to bf16 (halves out-DMA bytes; ~0.1% rounding rides in
                    # the error budget). DVE keeps the out path off the
                    # in-order ScalarE queue (busy on silus).
                    def emit_tail(outps=tuple(outps), ooff=ooff, n=n, t=t):
                        osb = osb_pool.tile(
                            [128, OC, n], BF16, name="osb", tag="osb"
                        )
                        final_scale = 1.0 / SCALE_OUT if ROUTED_FP8 else 1.0
                        for c in range(OC):
                            nc.vector.tensor_scalar(
                                osb[:, c, :],
                                outps[c][:, :n],
                                final_scale,
                                b2s_sb[:, c : c + 1],
                                mybir.AluOpType.mult,
                                mybir.AluOpType.add,
                            )
                        # alternate out transfers across the two DMA queues
                        # so neither's drain backs up at kernel end
                        eng = nc.sync if t % 2 == 0 else nc.gpsimd
                        eng.dma_start(
                            outT_d[:, ooff : ooff + OC * n], osb[:]
                        )

                    def emit_block(mm2=emit_mm2, tail=emit_tail, last=last_e):
                        mm2()
                        if last:
                            tail()

                    pending.append(emit_block)

                a0 += n
                xoff += KC * n
                woff += nr * n
                ooff += OC * n
            flush_pending()

    nc.compile()
    return nc


def _alpha_solve(rW2, rb2):
    """alpha_e = min-norm solution of W2_e @ alpha = b2_e. [6, HID]"""
    alphas = []
    for e in range(N_ROUTED):
        a, *_ = np.linalg.lstsq(rW2[e].astype(np.float64), rb2[e].astype(np.float64))
        alphas.append(a)
    return np.stack(alphas).astype(np.float32)


def _hid_permutation(b1):
    """Per-expert hidden-unit permutation pairing near-equal b1 values.

    Sorting units by b1 and pairing sorted neighbours into partner chunks
    (2P, 2P+1) lets ONE ScalarE activation cover a 2-chunk PSUM pair with a
    shared per-partition bias (the pair mean); the bias approximation error
    is ~1e-4 (W2-averaged to ~0.1% of out sigma), while halving ScalarE's
    per-instruction 352-cycle fixed cost.

    Returns (col_order [N_EXP, HID] old unit index per new column,
             b1pair [N_EXP, MC//2, 128] shared pair bias).
    """
    col_order = np.zeros((N_EXP, HID), dtype=np.int64)
    b1pair = np.zeros((N_EXP, MC // 2, 128), dtype=np.float32)
    p_idx = np.arange(128)
    for e in range(N_EXP):
        perm = np.argsort(b1[e], kind="stable")
        for c in range(MC):
            P, r = c // 2, c % 2
            col_order[e, c * 128 : (c + 1) * 128] = perm[2 * (128 * P + p_idx) + r]
        for P in range(MC // 2):
            u0 = col_order[e, (2 * P) * 128 : (2 * P + 1) * 128]
            u1 = col_order[e, (2 * P + 1) * 128 : (2 * P + 2) * 128]
            b1pair[e, P] = (b1[e, u0] + b1[e, u1]) / 2
    return col_order, b1pair


def _prep_host(inputs):
    feats = np.asarray(inputs["features"], dtype=np.float32)
    species = np.asarray(inputs["species_idx"]).astype(np.int64)
    emb = np.asarray(inputs["emb"], dtype=np.float32)
    Wr = np.asarray(inputs["W_router"], dtype=np.float32)
    rW1 = np.asarray(inputs["rW1"], dtype=np.float32)
    rb1 = np.asarray(inputs["rb1"], dtype=np.float32)
    rW2 = np.asarray(inputs["rW2"], dtype=np.float32)
    rb2 = np.asarray(inputs["rb2"], dtype=np.float32)
    sW1 = np.asarray(inputs["sW1"], dtype=np.float32)
    sb1 = np.asarray(inputs["sb1"], dtype=np.float32)
    sW2 = np.asarray(inputs["sW2"], dtype=np.float32)
    sb2 = np.asarray(inputs["sb2"], dtype=np.float32)

    wt_table, top2 = _router_table(emb, Wr)
    idx_cores, tiles = _plan_sharding(species, top2)
    nl = idx_cores.shape[1]
    # gate rows carry SCALE_G*w; on the decomp fp8 path h_sb is SCALE_W*h so
    # the gate instead carries SCALE_G/SCALE_W (hpm is identical either way)
    gf = SCALE_G
    if SILU_DECOMP and ROUTED_FP8:
        gf = SCALE_G / SCALE_W
    if not ROUTED_FP8:
        gf = 1.0
    w_atoms = wt_table[species] * gf  # [n, 6] f32

    b1 = np.concatenate([rb1, sb1], axis=0)  # [8, HID]
    col_order, b1pair = _hid_permutation(b1)

    W1 = np.concatenate([rW1, sW1], axis=0)   # [8, HID, IN_F]
    W2 = np.concatenate([rW2, sW2], axis=0)   # [8, OUT_F, HID]
    W1p = np.stack([W1[e][col_order[e]] for e in range(N_EXP)])
    W2p = np.stack([W2[e][:, col_order[e]] for e in range(N_EXP)])

    al = _alpha_solve(rW2, rb2)  # [6, HID], old unit order
    alp = np.stack([al[e][col_order[e]] for e in range(N_ROUTED)])
    if SILU_DECOMP and ROUTED_FP8:
        alp = alp * SCALE_W  # h_sb carries SCALE_W*h on the decomp path
    alpha_packed = np.ascontiguousarray(
        alp.reshape(N_ROUTED, MC, 128).transpose(2, 0, 1).reshape(128, N_ROUTED * MC)
    )

    # decomp rank-1 bias rows use the SAME pair-mean bias as the HW
    # activation so CoreSim validates the pairing approximation
    b1bar = np.repeat(b1pair, 2, axis=1).reshape(N_EXP, HID)
    b1_scaled = b1bar.copy()
    if SILU_DECOMP and ROUTED_FP8:
        b1_scaled[:N_ROUTED] *= SCALE_W  # rank-1 bias lands in the x32 psum

    def pack_w(w, kc, cols):
        # [E, rows=kc*128, cols] -> [E, 128, kc, cols] contiguous per expert
        e = w.shape[0]
        return np.ascontiguousarray(
            w.reshape(e, kc, 128, cols).transpose(0, 2, 1, 3)
        )

    w1sT = pack_w(W1p[N_ROUTED:].transpose(0, 2, 1), KC, HID).astype(BF16_NP)
    w2s_scale = SCALE_OUT if ROUTED_FP8 else 1.0
    w2sT = pack_w(
        w2s_scale * W2p[N_ROUTED:].transpose(0, 2, 1), MC, OUT_F
    ).astype(BF16_NP)
    if ROUTED_FP8:
        w1qT = pack_w(
            SCALE_W * W1p[:N_ROUTED].transpose(0, 2, 1), KC, HID
        ).astype(FP8_NP)
        w2qT = pack_w(
            SCALE_W * W2p[:N_ROUTED].transpose(0, 2, 1), MC, OUT_F
        ).astype(FP8_NP)
    else:
        w1qT = pack_w(W1p[:N_ROUTED].transpose(0, 2, 1), KC, HID).astype(BF16_NP)
        w2qT = pack_w(W2p[:N_ROUTED].transpose(0, 2, 1), MC, OUT_F).astype(BF16_NP)

    shared = {
        "w1q": w1qT,
        "w2q": w2qT,
        "w1s": w1sT,
        "w2s": w2sT,
        "b1": np.ascontiguousarray(
            b1pair.transpose(2, 0, 1).reshape(128, N_EXP * (MC // 2))
        ),
        "b1r": b1_scaled.reshape(1, N_EXP * HID).astype(BF16_NP),
        "alpha": alpha_packed,
        "b2s": np.ascontiguousarray(sb2.sum(axis=0).reshape(OC, 128).T),
    }

    in_maps = []
    for c in range(N_CORES):
        idx = idx_cores[c]
        valid = idx >= 0
        iv = idx[valid]
        xf = np.zeros((IN_F, nl), dtype=np.float32)
        xf[:, valid] = feats[iv].T
        # [128, KC, nl]: partition p + chunk k -> input feature k*128+p
        xv = np.ascontiguousarray(xf.reshape(KC, 128, nl).transpose(1, 0, 2))
        wfull = np.zeros((N_ROUTED, nl), dtype=np.float32)
        wfull[:, valid] = w_atoms[iv].T
        xb, wb = [], []
        a0 = 0
        for n, routed in tiles:
            xb.append(xv[:, :, a0 : a0 + n].reshape(128, KC * n))
            wb.append(wfull[list(routed), a0 : a0 + n].reshape(1, -1))
            a0 += n
        x_packed = np.concatenate(xb, axis=1)
        im = {
            "xT": x_packed.astype(BF16_NP),
            "w6": np.concatenate(wb, axis=1).astype(BF16_NP),
            **shared,
        }
        if ROUTED_FP8:
            im["xqT"] = x_packed.astype(FP8_NP)
        in_maps.append(im)
    return in_maps, idx_cores, tiles, nl, feats.shape[0]


_PROGRAM_CACHE = {}


def _get_program(nl, tiles):
    key = (nl, tuple(tiles), ROUTED_FP8, SILU_DECOMP, SILU_AS_SIGMOID)
    if key not in _PROGRAM_CACHE:
        _PROGRAM_CACHE[key] = _build_program(nl, tiles)
    return _PROGRAM_CACHE[key]


# Set TRACE=True (e.g. from a test harness) to capture a neuron-profile trace;
# the full BassKernelResults of the last run is kept in LAST_RESULTS.
TRACE = False
LAST_RESULTS = None


def kernel(**inputs):
    global LAST_RESULTS
    in_maps, idx_cores, tiles, nl, n_atoms = _prep_host(inputs)
    nc = _get_program(nl, tiles)
    res = run_bass_kernel_spmd(nc, in_maps, list(range(N_CORES)), trace=TRACE)
    LAST_RESULTS = res
    out = np.zeros((n_atoms, OUT_F), dtype=np.float32)
    for c in range(N_CORES):
        idx = idx_cores[c]
        valid = idx >= 0
        outT = res.results[c]["outT"]  # [128, tot_out] bf16 tile-packed
        rows = []
        off = 0
        for n, _ in tiles:
            blk = outT[:, off : off + OC * n].reshape(128, OC, n)
            # [n, OC*128] with out feature index c*128+p
            rows.append(blk.transpose(2, 1, 0).reshape(n, OUT_F))
            off += OC * n
        out_core = np.concatenate(rows, axis=0).astype(np.float32)
        out[idx[valid]] = out_core[valid]
    return out



# revision 14
# speedup vs baseline: 1.0401x; 1.0401x over previous
"""Trainium2 Bass kernel for a species-routed MoE readout layer.

Math (see problem reference): per atom x [512]:
  u = silu(emb[species]); scores = softmax(u @ Wr.T)  -> top-2 sparse gates
  out = sum_e gate_e * (W2_e @ silu(W1_e @ x + b1_e) + b2_e)
      + sum_s (W2_s @ silu(W1_s @ x + b1_s) + b2_s)          # 2 shared experts

The router depends only on species_idx (64 species), so the per-atom top-2
gates collapse to a host-computed 64x6 lookup table. Atoms are grouped by
their top-2 expert pair and each group is split evenly across the 8 cores so
the single SPMD program sees the same tile->active-expert pattern on every
core; interior tiles then only compute 2 routed + 2 shared expert MLPs
instead of all 8.

Precision split (the router logits are tiny, so every softmax gate is
~1/6: routed-expert outputs are damped 6x while the 2 shared experts pass
at full weight): the 2 routed experts run in fp8-e4m3 with DoubleRow
double-pumped matmuls (2 contraction planes per PE pass, ~2x bf16), the
2 shared experts stay bf16. Measured end-to-end max-rel error ~9e-3 vs
the 2e-2 budget (all-bf16 is 3.4e-3; all-fp8 would be 5e-2).

Scale plumbing (e4m3 normals bottom out at 2^-6, weights are ~0.02):
  W1q = e4m3(32*W1)  -> mm1 psum = 32*pre; silu applied as
        ScalarE activation(scale=1/32, bias=b1)
  hpm = e4m3((h + alpha_e) * 8w)  on DVE STT (w6 carries 8*gate;
        alpha_e = lstsq(W2_e, b2_e) folds the gated b2 into the gate mult)
  W2q = e4m3(32*W2)  -> routed psum2 terms = 256 * w*(W2_e(h+alpha))
  shared W2 (bf16) pre-scaled x256 to match; final PSUM->SBUF copy is
  (psum * 1/256) + sum_s b2_s on the DVE tensor_scalar.

Device side (per core, per <=512-atom variable-size tile aligned to
expert-pair segment boundaries), f32 PSUM accumulation throughout.
ScalarE's ACTIVATE carries a 352-cycle fixed cost, so each expert's
hidden units are host-permuted to pair near-equal b1 values into partner
chunks (2P, 2P+1): one silu activation then covers a 2-bank PSUM pair
with the shared pair-mean bias (approximation error ~3e-4 max-rel),
halving ScalarE's instruction count. mm2 blocks are software-pipelined
three expert-blocks behind mm1 so the in-order PE queue never waits on
the silu->gate-mult chain. A short PE spin + dummy activation at kernel
start warms the HAM clock gate (cold PE runs at 1.2 instead of 2.4 GHz)
and the ACT table while the first DMAs are in flight; per-expert weight
tiles stream on the sync queue in first-use order behind tile 0's
inputs, gate rows/consts/broadcasts ride the GPSIMD queue.
"""

import numpy as np
import ml_dtypes

import concourse.bass as bass
import concourse.mybir as mybir
from concourse import bacc, tile
from concourse.bass_utils import run_bass_kernel_spmd

BF16 = mybir.dt.bfloat16
FP8 = mybir.dt.float8e4
F32 = mybir.dt.float32
BF16_NP = ml_dtypes.bfloat16
FP8_NP = ml_dtypes.float8_e4m3  # IEEE e4m3 (max 240) == TRN FP8_EXP4

N_CORES = 8
N_ATOMS = 100000
IN_F = 512
HID = 512
OUT_F = 256
N_ROUTED = 6
N_SHARED = 2
N_EXP = N_ROUTED + N_SHARED
TOPK = 2
TILE_N = 512  # atoms per tile = one PSUM bank = max matmul moving dim
KC = IN_F // 128   # 4 contraction chunks for mm1
MC = HID // 128    # 4 hid chunks
OC = OUT_F // 128  # 2 out chunks

SCALE_W = 32.0           # fp8 weight scale (W1q and W2q)
SCALE_G = 8.0            # gate scale folded into the hpm quantization
SCALE_OUT = SCALE_W * SCALE_G  # routed+shared psum arrives x256

SPARSE = True  # compute only active routed experts per tile
ROUTED_FP8 = True  # routed experts in fp8 DoubleRow; False = all-bf16
# CoreSim has no Silu: decompose as x*sigmoid(x) (bias folded into PSUM via a
# rank-1 matmul). HW path uses native ScalarE Silu with the f32 bias operand.
SILU_DECOMP = False
# Timing-only CoreSim mode: emit the HW-path instruction stream but with
# Sigmoid in place of Silu (identical cost shape, wrong values).
SILU_AS_SIGMOID = False

DR = mybir.MatmulPerfMode.DoubleRow


def _silu(x):
    return x / (1.0 + np.exp(-x))


def _router_table(emb, W_router):
    """[64, 6] sparse top-2 gate table + per-species expert pair."""
    u = _silu(emb.astype(np.float32))
    logits = u @ W_router.astype(np.float32).T
    m = logits.max(axis=-1, keepdims=True)
    e = np.exp(logits - m)
    scores = e / e.sum(axis=-1, keepdims=True)
    order = np.argsort(-scores, axis=-1, kind="stable")
    top2 = order[:, :TOPK]
    wt = np.zeros_like(scores)
    rows = np.arange(scores.shape[0])[:, None]
    wt[rows, top2] = scores[rows, top2]
    return wt, top2


def _plan_sharding(species_idx, top2):
    """Group atoms by top-2 expert pair, split each group evenly over cores.

    Returns (idx_cores [N_CORES, NL] int64 with -1 padding, tiles) where
    tiles is a list of (n_atoms, active_routed_experts) per tile,
    identical for every core by construction.
    """
    n = species_idx.shape[0]
    if not SPARSE:
        assert n % N_CORES == 0
        nl = n // N_CORES
        idx_cores = np.arange(n, dtype=np.int64).reshape(N_CORES, nl)
        tiles = []
        for t0 in range(0, nl, TILE_N):
            tiles.append((min(TILE_N, nl - t0), tuple(range(N_ROUTED))))
        return idx_cores, tiles

    MIN_TILE = 64  # merge segments smaller than this into their neighbor

    pair_of_species = [tuple(sorted(top2[s])) for s in range(top2.shape[0])]
    pairs = sorted(set(pair_of_species))
    pair_id_of_species = np.array(
        [pairs.index(p) for p in pair_of_species], dtype=np.int64
    )
    atom_pair = pair_id_of_species[species_idx]

    seg_lens = []       # per-group per-core segment length
    seg_experts = []
    group_idx = []      # per-group atom index arrays
    for g, p in enumerate(pairs):
        idx_g = np.nonzero(atom_pair == g)[0]
        if idx_g.size == 0:
            continue
        L = -(-idx_g.size // N_CORES)  # ceil
        seg_lens.append(L)
        seg_experts.append(tuple(int(x) for x in p))
        group_idx.append(idx_g)

    # largest group first: deep pipeline while the clock warms, short tail
    order = np.argsort([-L for L in seg_lens], kind="stable")
    seg_lens = [seg_lens[i] for i in order]
    seg_experts = [seg_experts[i] for i in order]
    group_idx = [group_idx[i] for i in order]

    nl = sum(seg_lens)
    idx_cores = np.full((N_CORES, nl), -1, dtype=np.int64)
    off = 0
    for L, idx_g in zip(seg_lens, group_idx):
        for c in range(N_CORES):
            part = idx_g[c * L : (c + 1) * L]
            idx_cores[c, off : off + part.size] = part
        off += L

    # Variable-size tiles aligned to segment boundaries: each tile covers a
    # single expert pair (tiny segments merge into their neighbor).
    tiles = []
    pend_n, pend_e = 0, set()
    for L, p in zip(seg_lens, seg_experts):
        pend_n += L
        pend_e.update(p)
        if pend_n < MIN_TILE:
            continue
        k = -(-pend_n // TILE_N)
        q, r = divmod(pend_n, k)
        for i in range(k):
            tiles.append((q + (1 if i < r else 0), tuple(sorted(pend_e))))
        pend_n, pend_e = 0, set()
    if pend_n:
        if tiles:
            n0, e0 = tiles.pop()
            pend_n += n0
            pend_e.update(e0)
        k = -(-pend_n // TILE_N)
        q, r = divmod(pend_n, k)
        ee = tuple(sorted(pend_e))
        for i in range(k):
            tiles.append((q + (1 if i < r else 0), ee))
    assert sum(t[0] for t in tiles) == nl
    return idx_cores, tiles


def _build_program(nl, tiles):
    nc = bacc.Bacc("TRN2", target_bir_lowering=False, debug=False)

    # x/xq/w6/out are TILE-PACKED on the host: per tile a contiguous
    # [128, KC*n] (resp. [1, r*n] / [128, OC*n]) block, so every stream
    # DMA is one big contiguous transfer instead of 512 sub-KB rows.
    tot_x = sum(KC * n for n, _ in tiles)
    tot_w6 = sum(len(r) * n for n, r in tiles)
    tot_out = sum(OC * n for n, _ in tiles)
    xT_d = nc.declare_dram_parameter("xT", [128, tot_x], BF16, isOutput=False)
    if ROUTED_FP8:
        xqT_d = nc.declare_dram_parameter("xqT", [128, tot_x], FP8, isOutput=False)
    w6_d = nc.declare_dram_parameter("w6", [1, tot_w6], BF16, isOutput=False)
    rw_dt = FP8 if ROUTED_FP8 else BF16
    # weights contiguous per expert ([e][p][k][cols]) so the first
    # critical-path loads run at full DMA bandwidth
    w1q_d = nc.declare_dram_parameter("w1q", [N_ROUTED, 128, KC, HID], rw_dt, isOutput=False)
    w2q_d = nc.declare_dram_parameter("w2q", [N_ROUTED, 128, MC, OUT_F], rw_dt, isOutput=False)
    w1s_d = nc.declare_dram_parameter("w1s", [N_SHARED, 128, KC, HID], BF16, isOutput=False)
    w2s_d = nc.declare_dram_parameter("w2s", [N_SHARED, 128, MC, OUT_F], BF16, isOutput=False)
    b1_d = nc.declare_dram_parameter("b1", [128, N_EXP * (MC // 2)], F32, isOutput=False)
    b1r_d = nc.declare_dram_parameter("b1r", [1, N_EXP * HID], BF16, isOutput=False)
    # alpha[e] solves W2_e @ alpha_e = b2_e (host lstsq), so the gated b2
    # rides the gate multiply: W2_e @ (w*(h+alpha)) = w*(W2_e h) + w*b2_e
    alpha_d = nc.declare_dram_parameter(
        "alpha", [128, N_ROUTED * MC], F32, isOutput=False
    )
    b2s_d = nc.declare_dram_parameter("b2s", [128, OC], F32, isOutput=False)
    outT_d = nc.declare_dram_parameter("outT", [128, tot_out], BF16, isOutput=True)

    mm1_scale = 1.0 / SCALE_W if ROUTED_FP8 else 1.0

    with tile.TileContext(nc) as tc:
        with (
            tc.tile_pool(name="consts", bufs=1) as consts,
            tc.tile_pool(name="xp", bufs=6) as xp,
            tc.tile_pool(name="xqp", bufs=6) as xqp,
            tc.tile_pool(name="w6p", bufs=3) as w6p,
            tc.tile_pool(name="wbcp", bufs=6) as wbcp,
            tc.tile_pool(name="hps", bufs=2, space="PSUM") as hpsp,
            tc.tile_pool(name="hp", bufs=10) as hp_pool,
            tc.tile_pool(name="hpp", bufs=6) as hpp_pool,
            tc.tile_pool(name="ops", bufs=4, space="PSUM") as outps_pool,
            tc.tile_pool(name="osb", bufs=5) as osb_pool,
        ):
            # ---- constants / weights preload ----
            # Queue split: bulk streams (x/xq + expert weights) on the sync
            # queue; small/slow-lane transfers (per-partition consts, the
            # single-partition w6 gate rows) plus gate broadcasts and output
            # writes on the GPSIMD queue. Keeps tile0's first mm1 deps (w1[e0]
            # + xq) arriving ~1.7us in, right as the HAM warm-up spin ends.
            b1_sb = consts.tile([128, N_EXP * (MC // 2)], F32, name="b1_sb")
            alpha_sb = consts.tile([128, N_ROUTED * MC], F32, name="alpha_sb")
            b2s_sb = consts.tile([128, OC], F32, name="b2s_sb")
            ones_sb = consts.tile([1, 128], BF16, name="ones_sb")
            ones_row = consts.tile([1, TILE_N], BF16, name="ones_row")

            # consts ride the (otherwise idle) scalar queue so the gpsimd
            # queue's first transfer is tile0's gate row
            nc.scalar.dma_start(b1_sb[:], b1_d[:])
            nc.scalar.dma_start(alpha_sb[:], alpha_d[:])
            nc.scalar.dma_start(b2s_sb[:], b2s_d[:])
            if SILU_DECOMP:
                # dead in the HW path; its single-partition row is a ~3us
                # slow-lane DMA, so only load it for the CoreSim decomp
                b1row_sb = consts.tile([1, N_EXP * HID], BF16, name="b1row_sb")
                nc.gpsimd.dma_start(b1row_sb[:], b1r_d[:])
            nc.vector.memset(ones_sb[:], 1.0)
            nc.vector.memset(ones_row[:], 1.0)

            # Per-expert weight tiles, loaded in first-use order so tile 0's
            # matmuls start after the first expert's ~0.4MB instead of the
            # full weight set.
            # tile0 runs shared-first (its mm2 path needs no gate machinery,
            # so the w6->broadcast->STT chain warms in the background)
            eorder = []
            for t, (_, routed) in enumerate(tiles):
                shared_e = [N_ROUTED + s for s in range(N_SHARED)]
                order = (shared_e + list(routed)) if t == 0 else (
                    list(routed) + shared_e
                )
                for e in order:
                    if e not in eorder:
                        eorder.append(e)
                if len(eorder) == N_EXP:
                    break
            for e in range(N_EXP):
                if e not in eorder:
                    eorder.append(e)

            # one contiguous DMA per expert per matrix (queue-issue cost is
            # per-op, transfers run on the parallel DMA engines)
            w1_sb = {}
            w2_sb = {}

            def load_w1(e):
                dt = rw_dt if e < N_ROUTED else BF16
                w1_sb[e] = consts.tile([128, KC, HID], dt, name=f"w1e{e}")
                src = w1q_d[e] if e < N_ROUTED else w1s_d[e - N_ROUTED]
                nc.sync.dma_start(w1_sb[e][:], src)

            def load_w2(e):
                dt = rw_dt if e < N_ROUTED else BF16
                w2_sb[e] = consts.tile([128, MC, OUT_F], dt, name=f"w2e{e}")
                src = w2q_d[e] if e < N_ROUTED else w2s_d[e - N_ROUTED]
                nc.sync.dma_start(w2_sb[e][:], src)

            # only w1[e0] ahead of tile0's xq: first mm1 needs exactly these
            load_w1(eorder[0])

            # Warm the PE HAM clock gate (cold = 1.2 GHz until ~3.4us of
            # sustained activity) and the ScalarE activation table while the
            # first input DMAs are in flight.
            warm_sb = consts.tile([128, 1], F32, name="warm_sb")
            for _ in range(8):
                warm_ps = hpsp.tile(
                    [128, 2, TILE_N], F32, name="warm_ps", tag="hps"
                )
                nc.tensor.matmul(
                    warm_ps[:, 0, :], ones_sb[:, :], ones_row[0:1, :],
                    start=True, stop=True,
                )
            nc.scalar.activation(
                warm_sb[:, :], b1_sb[:, 0:1],
                mybir.ActivationFunctionType.Sigmoid
                if (SILU_DECOMP or SILU_AS_SIGMOID)
                else mybir.ActivationFunctionType.Silu,
            )

            # ---- main loop over atom tiles (tile-packed dram offsets) ----
            xoff = 0
            woff = 0
            ooff = 0

            # Software pipelining: each expert's mm2 (and, for a tile's last
            # expert, the psum->sbuf tail) is emitted three expert-blocks
            # late (flush keeps 2 pending), so the in-order PE queue never
            # stalls on the ScalarE silu -> DVE gate-mult chain of the last
            # m-chunk (one block of fp8 mm1 is shorter than that latency).
            pending = []

            def flush_pending(keep=0):
                while len(pending) > keep:
                    pending.pop(0)()

            a0 = 0
            spare_experts = []
            for t, (n, routed) in enumerate(tiles):
                # alternate routed (fp8, short mm1 bursts) with shared (bf16,
                # long mm1) so ScalarE's silu drain keeps up with PSUM-bank
                # production; tile0 runs routed first so its startup deps are
                # the cheap fp8 streams (xq + fp8 w1) only
                rr = list(routed)
                ss = [N_ROUTED + s for s in range(N_SHARED)]
                if t == 0:
                    experts = ss + rr
                else:
                    experts = []
                    while rr or ss:
                        if rr:
                            experts.append(rr.pop(0))
                        if ss:
                            experts.append(ss.pop(0))
                # safety net: a tile must never route to an unloaded expert
                for e in experts:
                    if t > 0 and e not in w1_sb:
                        if e in spare_experts:
                            spare_experts.remove(e)
                        load_w1(e)
                        load_w2(e)

                # gate rows packed onto partition 0 (single-partition DMAs are
                # the slow per-partition lane -> GPSIMD queue; one contiguous
                # block with this tile's routed rows only)
                nr = len(routed)
                w6row = w6p.tile([1, nr, n], BF16, name="w6row", tag="w6r")
                x_sb = xp.tile([128, KC, n], BF16, name="x_sb", tag="x")
                if ROUTED_FP8:
                    xq_sb = xqp.tile([128, KC, n], FP8, name="xq_sb", tag="xq")
                if t == 0:
                    # shared-first tile0: the bf16 x stream + shared mm2
                    # weights lead; the fp8 stream and gate row follow with
                    # ~4 expert-blocks of slack before the first routed mm2
                    nc.gpsimd.dma_start(
                        w6row[:], w6_d[0:1, woff : woff + nr * n]
                    )
                    nc.sync.dma_start(x_sb[:], xT_d[:, xoff : xoff + KC * n])
                    load_w2(experts[0])
                    load_w1(experts[1])
                    load_w2(experts[1])
                    if ROUTED_FP8:
                        nc.sync.dma_start(
                            xq_sb[:], xqT_d[:, xoff : xoff + KC * n]
                        )
                else:
                    # xq first: the routed experts run first in the interleave
                    if ROUTED_FP8:
                        nc.sync.dma_start(
                            xq_sb[:], xqT_d[:, xoff : xoff + KC * n]
                        )
                    nc.gpsimd.dma_start(
                        w6row[:], w6_d[0:1, woff : woff + nr * n]
                    )
                    nc.sync.dma_start(x_sb[:], xT_d[:, xoff : xoff + KC * n])

                if t == 0:
                    # tile0's own remaining weights right behind its inputs;
                    # spare experts trickle one per later tile so they never
                    # sit ahead of the next tiles' x/xq streams
                    for e in experts[2:]:
                        if e not in w1_sb:
                            load_w1(e)
                        if e not in w2_sb:
                            load_w2(e)
                    spare_experts.extend(
                        e for e in eorder if e not in w1_sb
                    )
                elif spare_experts:
                    e_sp = spare_experts.pop(0)
                    if e_sp not in w1_sb:
                        load_w1(e_sp)
                        load_w2(e_sp)

                # per-atom gates broadcast across 128 partitions (GPSIMD,
                # keeps PE free)
                wsb = {}
                for si, e in enumerate(routed):
                    wsb_e = wbcp.tile([128, TILE_N], BF16, name="wsb", tag="wbc")
                    nc.gpsimd.partition_broadcast(
                        wsb_e[:, :n], w6row[0:1, si, :n]
                    )
                    wsb[e] = wsb_e

                # output accumulators
                outps = [
                    outps_pool.tile([128, TILE_N], F32, name="ops", tag="ops")
                    for _ in range(OC)
                ]

                for ei, e in enumerate(experts):
                    is_routed = e < N_ROUTED
                    use_fp8 = ROUTED_FP8 and is_routed
                    # DoubleRow only beats plain matmul when the stream
                    # (1.13*n/2 cyc/chunk) outruns its 256-col LDW; below
                    # n~256 use mixed fp8-weight x bf16-x matmuls instead
                    # (same fp8 weights and scales, bf16 gate path)
                    use_dr = use_fp8 and n >= 256
                    last_e = ei == len(experts) - 1
                    # flush the 3-blocks-ago mm2 BEFORE this expert's mm1:
                    # under the routed/shared interleave the flushed block is
                    # the opposite precision, so DoubleRow bursts (whose
                    # LDWEIGHTS chain outpaces their streams) are separated
                    # by bf16 work that lets the weight port drain
                    flush_pending(keep=2)
                    hpm8 = None
                    if use_dr:
                        hpm8 = hpp_pool.tile(
                            [128, MC, TILE_N], FP8, name="hpm8", tag="hpm8"
                        )
                    hpm_list = []  # (pair_tile, r) mm2 rhs per m-chunk (bf16)
                    for P in range(MC // 2):
                        # 2-chunk PSUM pair: partner chunks 2P/2P+1 share a
                        # per-partition bias (host pairs near-equal b1), so
                        # ONE ScalarE activation covers both banks.
                        hps = hpsp.tile(
                            [128, 2, TILE_N], F32, name="hps", tag="hps"
                        )
                        for r in range(2):
                            m = 2 * P + r
                            if use_dr:
                                for kk in range(0, KC, 2):
                                    nc.tensor.matmul(
                                        hps[:, r, :n],
                                        w1_sb[e][:, kk : kk + 2, m * 128 : (m + 1) * 128],
                                        xq_sb[:, kk : kk + 2, :n],
                                        start=(kk == 0),
                                        stop=(kk == KC - 2 and not SILU_DECOMP),
                                        perf_mode=DR,
                                    )
                            else:
                                for k in range(KC):
                                    nc.tensor.matmul(
                                        hps[:, r, :n],
                                        w1_sb[e][:, k, m * 128 : (m + 1) * 128],
                                        x_sb[:, k, :n],
                                        start=(k == 0),
                                        stop=(k == KC - 1 and not SILU_DECOMP),
                                    )
                            if SILU_DECOMP:
                                boff = e * HID + m * 128
                                nc.tensor.matmul(
                                    hps[:, r, :n],
                                    b1row_sb[0:1, boff : boff + 128],
                                    ones_row[0:1, :n],
                                    start=False, stop=True,
                                )
                        h_sb = hp_pool.tile(
                            [128, 2, TILE_N], BF16, name="h_sb", tag="h"
                        )
                        sc = mm1_scale if use_fp8 else 1.0
                        if SILU_DECOMP:
                            # bias is already in psum (rank-1, pair-mean b1);
                            # h_sb ends up as sc^-1 * h, absorbed by STT/w2
                            s_sb = hp_pool.tile(
                                [128, 2, TILE_N], BF16, name="s_sb", tag="s"
                            )
                            nc.scalar.activation(
                                s_sb[:, :, :n], hps[:, :, :n],
                                mybir.ActivationFunctionType.Sigmoid,
                                scale=sc,
                            )
                            nc.vector.tensor_mul(
                                h_sb[:, :, :n], hps[:, :, :n], s_sb[:, :, :n]
                            )
                        else:
                            nc.scalar.activation(
                                h_sb[:, :, :n], hps[:, :, :n],
                                mybir.ActivationFunctionType.Sigmoid
                                if SILU_AS_SIGMOID
                                else mybir.ActivationFunctionType.Silu,
                                bias=b1_sb[:, e * (MC // 2) + P : e * (MC // 2) + P + 1],
                                scale=sc,
                            )
                        for r in range(2):
                            m = 2 * P + r
                            if is_routed:
                                ac = e * MC + m
                                if use_dr:
                                    nc.vector.scalar_tensor_tensor(
                                        hpm8[:, m, :n],
                                        h_sb[:, r, :n],
                                        alpha_sb[:, ac : ac + 1],
                                        wsb[e][:, :n],
                                        mybir.AluOpType.add,
                                        mybir.AluOpType.mult,
                                    )
                                else:
                                    hpm = hpp_pool.tile(
                                        [128, TILE_N], BF16, name="hpm", tag="hpm"
                                    )
                                    nc.vector.scalar_tensor_tensor(
                                        hpm[:, :n],
                                        h_sb[:, r, :n],
                                        alpha_sb[:, ac : ac + 1],
                                        wsb[e][:, :n],
                                        mybir.AluOpType.add,
                                        mybir.AluOpType.mult,
                                    )
                                    hpm_list.append((hpm, None))
                            else:
                                hpm_list.append((h_sb, r))

                    def emit_mm2(
                        e=e, ei=ei, n=n, last_e=last_e, use_dr=use_dr,
                        hpm8=hpm8, hpm_list=tuple(hpm_list), outps=tuple(outps),
                    ):
                        if use_dr:
                            for c in range(OC):
                                for mm in range(0, MC, 2):
                                    nc.tensor.matmul(
                                        outps[c][:, :n],
                                        w2_sb[e][:, mm : mm + 2, c * 128 : (c + 1) * 128],
                                        hpm8[:, mm : mm + 2, :n],
                                        start=(ei == 0 and mm == 0),
                                        stop=(last_e and mm == MC - 2),
                                        perf_mode=DR,
                                    )
                        else:
                            for c in range(OC):
                                for m in range(MC):
                                    ht, hr = hpm_list[m]
                                    rhs = ht[:, :n] if hr is None else ht[:, hr, :n]
                                    nc.tensor.matmul(
                                        outps[c][:, :n],
                                        w2_sb[e][:, m, c * 128 : (c + 1) * 128],
                                        rhs,
                                        start=(ei == 0 and m == 0),
                                        stop=(last_e and m == MC - 1),
                                    )

                    # psum -> sbuf on DVE: (psum * 1/256) + sum_s b2_s, cast
                    # to bf16 (halves out-DMA bytes; ~0.1% rounding rides in
                    # the error budget). DVE keeps the out path off the
                    # in-order ScalarE queue (busy on silus).
                    def emit_tail(outps=tuple(outps), ooff=ooff, n=n, t=t):
                        osb = osb_pool.tile(
                            [128, OC, n], BF16, name="osb", tag="osb"
                        )
                        final_scale = 1.0 / SCALE_OUT if ROUTED_FP8 else 1.0
                        for c in range(OC):
                            nc.vector.tensor_scalar(
                                osb[:, c, :],
                                outps[c][:, :n],
                                final_scale,
                                b2s_sb[:, c : c + 1],
                                mybir.AluOpType.mult,
                                mybir.AluOpType.add,
                            )
                        # outputs ride the sync queue (idle after the startup
                        # weight burst; the GPSIMD queue pays a ~2us drain
                        # penalty on the final transfer)
                        nc.sync.dma_start(
                            outT_d[:, ooff : ooff + OC * n], osb[:]
                        )

                    def emit_block(mm2=emit_mm2, tail=emit_tail, last=last_e):
                        mm2()
                        if last:
                            tail()

                    pending.append(emit_block)

                a0 += n
                xoff += KC * n
                woff += nr * n
                ooff += OC * n
            flush_pending()

    nc.compile()
    return nc


def _alpha_solve(rW2, rb2):
    """alpha_e = min-norm solution of W2_e @ alpha = b2_e. [6, HID]"""
    alphas = []
    for e in range(N_ROUTED):
        a, *_ = np.linalg.lstsq(rW2[e].astype(np.float64), rb2[e].astype(np.float64))
        alphas.append(a)
    return np.stack(alphas).astype(np.float32)


def _hid_permutation(b1):
    """Per-expert hidden-unit permutation pairing near-equal b1 values.

    Sorting units by b1 and pairing sorted neighbours into partner chunks
    (2P, 2P+1) lets ONE ScalarE activation cover a 2-chunk PSUM pair with a
    shared per-partition bias (the pair mean); the bias approximation error
    is ~1e-4 (W2-averaged to ~0.1% of out sigma), while halving ScalarE's
    per-instruction 352-cycle fixed cost.

    Returns (col_order [N_EXP, HID] old unit index per new column,
             b1pair [N_EXP, MC//2, 128] shared pair bias).
    """
    col_order = np.zeros((N_EXP, HID), dtype=np.int64)
    b1pair = np.zeros((N_EXP, MC // 2, 128), dtype=np.float32)
    p_idx = np.arange(128)
    for e in range(N_EXP):
        perm = np.argsort(b1[e], kind="stable")
        for c in range(MC):
            P, r = c // 2, c % 2
            col_order[e, c * 128 : (c + 1) * 128] = perm[2 * (128 * P + p_idx) + r]
        for P in range(MC // 2):
            u0 = col_order[e, (2 * P) * 128 : (2 * P + 1) * 128]
            u1 = col_order[e, (2 * P + 1) * 128 : (2 * P + 2) * 128]
            b1pair[e, P] = (b1[e, u0] + b1[e, u1]) / 2
    return col_order, b1pair


def _prep_host(inputs):
    feats = np.asarray(inputs["features"], dtype=np.float32)
    species = np.asarray(inputs["species_idx"]).astype(np.int64)
    emb = np.asarray(inputs["emb"], dtype=np.float32)
    Wr = np.asarray(inputs["W_router"], dtype=np.float32)
    rW1 = np.asarray(inputs["rW1"], dtype=np.float32)
    rb1 = np.asarray(inputs["rb1"], dtype=np.float32)
    rW2 = np.asarray(inputs["rW2"], dtype=np.float32)
    rb2 = np.asarray(inputs["rb2"], dtype=np.float32)
    sW1 = np.asarray(inputs["sW1"], dtype=np.float32)
    sb1 = np.asarray(inputs["sb1"], dtype=np.float32)
    sW2 = np.asarray(inputs["sW2"], dtype=np.float32)
    sb2 = np.asarray(inputs["sb2"], dtype=np.float32)

    wt_table, top2 = _router_table(emb, Wr)
    idx_cores, tiles = _plan_sharding(species, top2)
    nl = idx_cores.shape[1]
    # gate rows carry SCALE_G*w; on the decomp fp8 path h_sb is SCALE_W*h so
    # the gate instead carries SCALE_G/SCALE_W (hpm is identical either way)
    gf = SCALE_G
    if SILU_DECOMP and ROUTED_FP8:
        gf = SCALE_G / SCALE_W
    if not ROUTED_FP8:
        gf = 1.0
    w_atoms = wt_table[species] * gf  # [n, 6] f32

    b1 = np.concatenate([rb1, sb1], axis=0)  # [8, HID]
    col_order, b1pair = _hid_permutation(b1)

    W1 = np.concatenate([rW1, sW1], axis=0)   # [8, HID, IN_F]
    W2 = np.concatenate([rW2, sW2], axis=0)   # [8, OUT_F, HID]
    W1p = np.stack([W1[e][col_order[e]] for e in range(N_EXP)])
    W2p = np.stack([W2[e][:, col_order[e]] for e in range(N_EXP)])

    al = _alpha_solve(rW2, rb2)  # [6, HID], old unit order
    alp = np.stack([al[e][col_order[e]] for e in range(N_ROUTED)])
    if SILU_DECOMP and ROUTED_FP8:
        alp = alp * SCALE_W  # h_sb carries SCALE_W*h on the decomp path
    alpha_packed = np.ascontiguousarray(
        alp.reshape(N_ROUTED, MC, 128).transpose(2, 0, 1).reshape(128, N_ROUTED * MC)
    )

    # decomp rank-1 bias rows use the SAME pair-mean bias as the HW
    # activation so CoreSim validates the pairing approximation
    b1bar = np.repeat(b1pair, 2, axis=1).reshape(N_EXP, HID)
    b1_scaled = b1bar.copy()
    if SILU_DECOMP and ROUTED_FP8:
        b1_scaled[:N_ROUTED] *= SCALE_W  # rank-1 bias lands in the x32 psum

    def pack_w(w, kc, cols):
        # [E, rows=kc*128, cols] -> [E, 128, kc, cols] contiguous per expert
        e = w.shape[0]
        return np.ascontiguousarray(
            w.reshape(e, kc, 128, cols).transpose(0, 2, 1, 3)
        )

    w1sT = pack_w(W1p[N_ROUTED:].transpose(0, 2, 1), KC, HID).astype(BF16_NP)
    w2s_scale = SCALE_OUT if ROUTED_FP8 else 1.0
    w2sT = pack_w(
        w2s_scale * W2p[N_ROUTED:].transpose(0, 2, 1), MC, OUT_F
    ).astype(BF16_NP)
    if ROUTED_FP8:
        w1qT = pack_w(
            SCALE_W * W1p[:N_ROUTED].transpose(0, 2, 1), KC, HID
        ).astype(FP8_NP)
        w2qT = pack_w(
            SCALE_W * W2p[:N_ROUTED].transpose(0, 2, 1), MC, OUT_F
        ).astype(FP8_NP)
    else:
        w1qT = pack_w(W1p[:N_ROUTED].transpose(0, 2, 1), KC, HID).astype(BF16_NP)
        w2qT = pack_w(W2p[:N_ROUTED].transpose(0, 2, 1), MC, OUT_F).astype(BF16_NP)

    shared = {
        "w1q": w1qT,
        "w2q": w2qT,
        "w1s": w1sT,
        "w2s": w2sT,
        "b1": np.ascontiguousarray(
            b1pair.transpose(2, 0, 1).reshape(128, N_EXP * (MC // 2))
        ),
        "b1r": b1_scaled.reshape(1, N_EXP * HID).astype(BF16_NP),
        "alpha": alpha_packed,
        "b2s": np.ascontiguousarray(sb2.sum(axis=0).reshape(OC, 128).T),
    }

    in_maps = []
    for c in range(N_CORES):
        idx = idx_cores[c]
        valid = idx >= 0
        iv = idx[valid]
        xf = np.zeros((IN_F, nl), dtype=np.float32)
        xf[:, valid] = feats[iv].T
        # [128, KC, nl]: partition p + chunk k -> input feature k*128+p
        xv = np.ascontiguousarray(xf.reshape(KC, 128, nl).transpose(1, 0, 2))
        wfull = np.zeros((N_ROUTED, nl), dtype=np.float32)
        wfull[:, valid] = w_atoms[iv].T
        xb, wb = [], []
        a0 = 0
        for n, routed in tiles:
            xb.append(xv[:, :, a0 : a0 + n].reshape(128, KC * n))
            wb.append(wfull[list(routed), a0 : a0 + n].reshape(1, -1))
            a0 += n
        x_packed = np.concatenate(xb, axis=1)
        im = {
            "xT": x_packed.astype(BF16_NP),
            "w6": np.concatenate(wb, axis=1).astype(BF16_NP),
            **shared,
        }
        if ROUTED_FP8:
            im["xqT"] = x_packed.astype(FP8_NP)
        in_maps.append(im)
    return in_maps, idx_cores, tiles, nl, feats.shape[0]


_PROGRAM_CACHE = {}


def _get_program(nl, tiles):
    key = (nl, tuple(tiles), ROUTED_FP8, SILU_DECOMP, SILU_AS_SIGMOID)
    if key not in _PROGRAM_CACHE:
        _PROGRAM_CACHE[key] = _build_program(nl, tiles)
    return _PROGRAM_CACHE[key]


# Set TRACE=True (e.g. from a test harness) to capture a neuron-profile trace;
# the full BassKernelResults of the last run is kept in LAST_RESULTS.
TRACE = False
LAST_RESULTS = None


def kernel(**inputs):
    global LAST_RESULTS
    in_maps, idx_cores, tiles, nl, n_atoms = _prep_host(inputs)
    nc = _get_program(nl, tiles)
    res = run_bass_kernel_spmd(nc, in_maps, list(range(N_CORES)), trace=TRACE)
    LAST_RESULTS = res
    out = np.zeros((n_atoms, OUT_F), dtype=np.float32)
    for c in range(N_CORES):
        idx = idx_cores[c]
        valid = idx >= 0
        outT = res.results[c]["outT"]  # [128, tot_out] bf16 tile-packed
        rows = []
        off = 0
        for n, _ in tiles:
            blk = outT[:, off : off + OC * n].reshape(128, OC, n)
            # [n, OC*128] with out feature index c*128+p
            rows.append(blk.transpose(2, 1, 0).reshape(n, OUT_F))
            off += OC * n
        out_core = np.concatenate(rows, axis=0).astype(np.float32)
        out[idx[valid]] = out_core[valid]
    return out

